# revision 4
# baseline (speedup 1.0000x reference)
"""HOPEBlock Trainium2 kernel v2 — 8-core hybrid (2-way batch x 4-way attention
head / token parallel).

Core c = (g, r): g = c // 4 (batch element), r = c % 4.
Per core: attention for heads [4r, 4r+4) of batch g over all tokens; after the
out-proj AllReduce, the MLP + RMSNorm/upd/sc run token-parallel on token chunk
r (512 tokens) with FULL fc1/fc2 weights (no second collective).

Instruction-count-lean design: one 4-bank-psum exp per 2 s-chunks, softmax
normalize via cross-partition reciprocal + partition_broadcast (3 ops/head),
biases folded into activation ops, norm_w folded into upd_w on the host.
"""

import numpy as np
import ml_dtypes
from contextlib import ExitStack

import concourse.bass as bass
import concourse.tile as tile
from concourse import bacc, mybir, library_config
from concourse.bass_utils import run_bass_kernel_spmd

F32 = mybir.dt.float32
BF16 = mybir.dt.bfloat16
AF = mybir.ActivationFunctionType
OP = mybir.AluOpType

B, S, H = 2, 2048, 1024
HEADS, HD = 16, 64
INNER = 4 * H
NCORES, TPW = 8, 4
HL = HEADS // TPW           # 4 local heads
SC = S // 128               # 16 s-chunks
TCH = S // TPW              # 512 tokens per final chunk
ROPE_THETA = 10000.0
RMS_EPS = 1.1920929e-07
RG = [[0, 1, 2, 3], [4, 5, 6, 7]]

NP_BF16 = ml_dtypes.bfloat16

_cached = {}


def build_program(reps=1, no_coll=False, phases="ABCDEF"):
    key = ("k", reps, no_coll, phases)
    if key in _cached:
        return _cached[key]
    nc = bacc.Bacc("TRN2", target_bir_lowering=False, debug=False,
                   num_devices=NCORES)

    def din(name, shape, dt=BF16):
        return nc.dram_tensor(name, shape, dt, kind="ExternalInput")

    xt = din("xt", [H, S])                 # x[g].T feature-major bf16
    xc = din("xc", [H, TCH], F32)          # x[g][:, token chunk].T fp32
    qkt = din("qkt", [H, 4 * 128])         # [qp0,qp1,kp0,kp1] col blocks
    vwt = din("vwt", [H, HL * HD])         # v weightsT head-major
    owt = din("owt", [HL * HD, H])         # out_w sliceT
    fc1t = din("fc1t", [H, INNER])         # full fc1T [feat, inner]
    fc1b = din("fc1b", [128, 32], F32)
    fc2t = din("fc2t", [INNER, H])         # full fc2T [inner, out]
    fc2b = din("fc2b", [128, 8], F32)
    updt = din("updt", [H, H])             # (upd_w * norm_w).T
    updb = din("updb", [128, 8], F32)
    sct = din("sct", [H, H])
    scb = din("scb", [128, 8], F32)
    cosf = din("cosf", [128, S])
    sinf = din("sinf", [128, S])
    out = nc.dram_tensor("out", [H, TCH], F32, kind="ExternalOutput")

    with tile.TileContext(nc) as tc:
        for _rep in range(reps):
            _emit_iter(nc, tc, xt, xc, qkt, vwt, owt, fc1t, fc1b, fc2t, fc2b,
                       updt, updb, sct, scb, cosf, sinf, out, no_coll=no_coll,
                       phases=phases)

    nc.compile()
    _cached[key] = nc
    return nc


def _emit_iter(nc, tc, xt, xc, qkt, vwt, owt, fc1t, fc1b, fc2t, fc2b,
               updt, updb, sct, scb, cosf, sinf, out, no_coll=False,
               phases="ABCDEF"):
    # out-proj partials, token-chunk-major: rows (r*H + f), cols local token.
    # ReduceScatter over the flat buffer hands rank r quarter r = chunk r.
    ao_bounce = nc.dram_tensor([TPW * H, TCH], F32)
    ao_red = nc.dram_tensor([H, TCH], F32)

    with ExitStack() as ctx:
        persist = ctx.enter_context(tc.tile_pool(name="persist", bufs=1))
        xc_sb = persist.tile([128, 8, TCH], F32, tag="xc")
        nc.sync.dma_start(xc_sb[:], xc.ap().rearrange("(c p) t -> p c t", p=128))
        fc1b_sb = persist.tile([128, 32], F32, tag="fc1b")
        nc.sync.dma_start(fc1b_sb[:], fc1b.ap())
        fc2b_sb = persist.tile([128, 8], F32, tag="fc2b")
        nc.sync.dma_start(fc2b_sb[:], fc2b.ap())
        updb_sb = persist.tile([128, 8], F32, tag="updb")
        nc.sync.dma_start(updb_sb[:], updb.ap())
        scb_sb = persist.tile([128, 8], F32, tag="scb")
        nc.sync.dma_start(scb_sb[:], scb.ap())
        ones1_sb = persist.tile([128, 1], F32, tag="ones1")
        nc.vector.memset(ones1_sb[:], 1.0)
        eps_sb = persist.tile([1, 1], F32, tag="eps")
        nc.vector.memset(eps_sb[:], RMS_EPS)

        onpool = ctx.enter_context(tc.tile_pool(name="onpool", bufs=1))
        on_sb = onpool.tile([128, 2, S], BF16, tag="on")

        with tc.tile_pool(name="cpool", bufs=1) as cpool:
            x_sb = cpool.tile([128, 8, S], BF16, tag="x")
            nc.sync.dma_start(x_sb[:], xt.ap().rearrange("(c p) t -> p c t", p=128))
            q_sb = cpool.tile([128, 2, S], BF16, tag="q")
            k_sb = cpool.tile([128, 2, S], BF16, tag="k")
            # vt0: [v_h(2j) | ones]  (out rows 0-63, denom row 64)
            # vt1: [ones | junk | v_h(2j+1)] (denom row 0, out rows 64-127)
            vt0_sb = cpool.tile([128, SC, 2, 65], BF16, tag="vt0")
            vt1_sb = cpool.tile([128, SC, 2, 128], BF16, tag="vt1")

            # ---------------- Phase A: QKV projections ----------------
            with tc.tile_pool(name="apool", bufs=1) as apool, \
                 tc.tile_pool(name="qkpsum", bufs=2, space="PSUM") as qkpsum, \
                 tc.tile_pool(name="vpsum", bufs=4, space="PSUM") as vpsum:
                qkt_sb = apool.tile([128, 8, 4 * 128], BF16, tag="qkt")
                nc.sync.dma_start(qkt_sb[:], qkt.ap().rearrange("(c p) m -> p c m", p=128))
                vwt_sb = apool.tile([128, 8, HL * HD], BF16, tag="vwt")
                nc.sync.dma_start(vwt_sb[:], vwt.ap().rearrange("(c p) m -> p c m", p=128))
                nc.vector.memset(vt0_sb[:, :, :, 64], 1.0)
                nc.vector.memset(vt1_sb[:, :, :, 0:64], 0.0)
                nc.vector.memset(vt1_sb[:, :, :, 0], 1.0)

                for mk in range(2):  # 0: q pairs, 1: k pairs
                    dst = q_sb if mk == 0 else k_sb
                    for t in range(4):
                        ps = qkpsum.tile([128, 2, 512], F32, tag="qkps",
                                         name=f"qk{mk}_{t}")
                        for f in range(8):
                            for j in range(2):
                                m = 2 * mk + j
                                nc.tensor.matmul(
                                    ps[:, j, :],
                                    qkt_sb[:, f, m * 128:(m + 1) * 128],
                                    x_sb[:, f, t * 512:(t + 1) * 512],
                                    start=(f == 0), stop=(f == 7))
                        nc.scalar.copy(dst[:, :, t * 512:(t + 1) * 512], ps[:])

                for sp in range(SC // 2):
                    pss = [vpsum.tile([128, HL * HD], F32, tag="vps",
                                      name=f"v{2 * sp + i}") for i in range(2)]
                    for f in range(8):
                        for i in range(2):
                            s = 2 * sp + i
                            nc.tensor.matmul(
                                pss[i][:],
                                x_sb[:, f, s * 128:(s + 1) * 128],
                                vwt_sb[:, f, :],
                                start=(f == 0), stop=(f == 7))
                    for i in range(2):
                        s = 2 * sp + i
                        pv = pss[i][:].rearrange("p (j l d) -> p j l d", j=2, l=2)
                        if i == 0:
                            nc.vector.tensor_copy(vt0_sb[:, s, :, 0:64], pv[:, :, 0, :])
                            nc.scalar.copy(vt1_sb[:, s, :, 64:128], pv[:, :, 1, :])
                        else:
                            nc.scalar.copy(vt0_sb[:, s, :, 0:64], pv[:, :, 0, :])
                            nc.vector.tensor_copy(vt1_sb[:, s, :, 64:128], pv[:, :, 1, :])

            # ---------------- Phase B: RoPE on q, k ----------------
            with tc.tile_pool(name="rpool", bufs=1) as rpool:
                cos_sb = rpool.tile([128, S], BF16, tag="cos")
                nc.sync.dma_start(cos_sb[:], cosf.ap())
                sin_sb = rpool.tile([128, S], BF16, tag="sin")
                nc.sync.dma_start(sin_sb[:], sinf.ap())
                sgn_sb = rpool.tile([128, 1], F32, tag="sgn")
                for blk in range(4):
                    nc.vector.memset(sgn_sb[32 * blk:32 * (blk + 1), :],
                                     -1.0 if blk % 2 == 0 else 1.0)
                for tens in (q_sb, k_sb):
                    a_t = rpool.tile([128, 2, S], BF16, tag="ropeA")
                    b_t = rpool.tile([128, 2, S], BF16, tag="ropeB")
                    bs_t = rpool.tile([128, 2, S], BF16, tag="ropeBs")
                    cosb = cos_sb[:, None, :].broadcast_to([128, 2, S])
                    sinb = sin_sb[:, None, :].broadcast_to([128, 2, S])
                    nc.vector.tensor_tensor(a_t[:], tens[:], cosb, OP.mult)
                    nc.vector.tensor_tensor(b_t[:], tens[:], sinb, OP.mult)
                    for blk in range(4):
                        src = blk + 1 if blk % 2 == 0 else blk - 1
                        nc.sync.dma_start(
                            bs_t[32 * blk:32 * (blk + 1), :, :],
                            b_t[32 * src:32 * (src + 1), :, :])
                    nc.vector.scalar_tensor_tensor(
                        tens[:], bs_t[:], sgn_sb[:, 0:1], a_t[:], OP.mult, OP.add)

            # ---------------- Phase C: attention ----------------
            with tc.tile_pool(name="spsum", bufs=2, space="PSUM") as spsum, \
                 tc.tile_pool(name="avpsum", bufs=4, space="PSUM") as avpsum, \
                 tc.tile_pool(name="epool", bufs=3) as epool, \
                 tc.tile_pool(name="npool", bufs=4) as npool:
                for j in range(2):
                    for qc in range(4):
                        qs = slice(qc * 512, (qc + 1) * 512)
                        av0 = avpsum.tile([65, 512], F32, tag="av",
                                          name=f"av0_{j}_{qc}")
                        av1 = avpsum.tile([128, 512], F32, tag="av",
                                          name=f"av1_{j}_{qc}")
                        for s in range(SC):
                            ss = slice(s * 128, (s + 1) * 128)
                            sco = spsum.tile([128, 1024], F32, tag="sco",
                                             name=f"sco{j}_{qc}_{s}")
                            nc.tensor.matmul(
                                sco[:, 0:512],
                                k_sb[0:64, j, ss], q_sb[0:64, j, qs],
                                start=True, stop=True, tile_position=(0, 0))
                            nc.tensor.matmul(
                                sco[:, 512:1024],
                                k_sb[64:128, j, ss], q_sb[64:128, j, qs],
                                start=True, stop=True, tile_position=(64, 0))
                            e_t = epool.tile([128, 1024], BF16, tag="exp",
                                             name=f"e{j}_{qc}_{s}")
                            nc.scalar.activation(e_t[:], sco[:], AF.Exp)
                            nc.tensor.matmul(
                                av0[:], vt0_sb[:, s, j, :], e_t[:, 0:512],
                                start=(s == 0), stop=(s == SC - 1))
                            nc.tensor.matmul(
                                av1[:], vt1_sb[:, s, j, :], e_t[:, 512:1024],
                                start=(s == 0), stop=(s == SC - 1))
                        # normalize: 3 ops per head
                        rst = npool.tile([32, 2, 512], F32, tag="rst",
                                         name=f"r{j}_{qc}")
                        nc.vector.reciprocal(rst[0:1, 0, :], av0[64:65, :])
                        nc.vector.reciprocal(rst[0:1, 1, :], av1[0:1, :])
                        bc0 = npool.tile([128, 512], F32, tag="bc",
                                         name=f"b0_{j}_{qc}")
                        nc.gpsimd.partition_broadcast(bc0[:], rst[0:1, 0, :])
                        bc1 = npool.tile([128, 512], F32, tag="bc",
                                         name=f"b1_{j}_{qc}")
                        nc.gpsimd.partition_broadcast(bc1[:], rst[0:1, 1, :])
                        nc.vector.tensor_tensor(
                            on_sb[0:64, j, qs], av0[0:64, :], bc0[0:64, :], OP.mult)
                        nc.vector.tensor_tensor(
                            on_sb[64:128, j, qs], av1[64:128, :], bc1[64:128, :],
                            OP.mult)

        if "D" not in phases:
            with tc.tile_pool(name="stub", bufs=1) as stub:
                st = stub.tile([128, 8, TCH], F32, tag="st")
                nc.scalar.copy(st[:], on_sb[:, 0, 0:TCH][:, None, :].broadcast_to([128, 8, TCH]))
                nc.sync.dma_start(out.ap().rearrange("(c p) t -> p c t", p=128), st[:])
            return
        # ---------------- Phase D: out-proj + AllReduce + h ----------------
        with tc.tile_pool(name="dpool", bufs=3) as dpool, \
             tc.tile_pool(name="dwpool", bufs=1) as dwpool, \
             tc.tile_pool(name="dpsum", bufs=2, space="PSUM") as dpsum:
            owt_sb = dwpool.tile([128, 2, H], BF16, tag="owt")
            nc.sync.dma_start(owt_sb[:], owt.ap().rearrange("(c p) o -> p c o", p=128))
            for oc in range(8):
                ao_t = dpool.tile([128, 4, 512], F32, tag="aot", name=f"aot{oc}")
                for th in range(2):
                    ps = dpsum.tile([128, 1024], F32, tag="aops",
                                    name=f"ao{oc}_{th}")
                    for c in range(2):
                        for i in range(2):
                            t = 2 * th + i
                            nc.tensor.matmul(
                                ps[:, i * 512:(i + 1) * 512],
                                owt_sb[:, c, oc * 128:(oc + 1) * 128],
                                on_sb[:, c, t * 512:(t + 1) * 512],
                                start=(c == 0), stop=(c == 1))
                    dst = ao_t[:, 2 * th:2 * th + 2, :]
                    if (oc + th) % 2 == 0:
                        nc.scalar.copy(dst, ps[:].rearrange("p (a b) -> p a b", b=512))
                    else:
                        nc.vector.tensor_copy(dst, ps[:].rearrange("p (a b) -> p a b", b=512))
                nc.sync.dma_start(
                    ao_bounce.ap().rearrange("(r c p) t -> c p r t",
                                             r=TPW, p=128)[oc], ao_t[:])
        if no_coll:
            nc.sync.dma_start(ao_red.ap(), ao_bounce.ap()[0:H, :])
        else:
            nc.gpsimd.collective_compute(
                "ReduceScatter", OP.add, replica_groups=RG,
                ins=[ao_bounce.ap()], outs=[ao_red.ap()])

        # h (bf16) for our 512-token chunk: xc (fp32) + reduced out-proj
        hpool = ctx.enter_context(tc.tile_pool(name="hpool", bufs=1))
        h_sb = hpool.tile([128, 8, TCH], BF16, tag="h")
        with tc.tile_pool(name="hstg", bufs=1) as hstg:
            ao_st = hstg.tile([128, 8, TCH], F32, tag="aost")
            nc.sync.dma_start(
                ao_st[:], ao_red.ap().rearrange("(c p) t -> p c t", p=128))
            nc.vector.tensor_tensor(h_sb[:], ao_st[:], xc_sb[:], OP.add)

        if "E" not in phases:
            with tc.tile_pool(name="stub2", bufs=1) as stub:
                st = stub.tile([128, 8, TCH], F32, tag="st2")
                nc.vector.tensor_copy(st[:], h_sb[:])
                nc.sync.dma_start(out.ap().rearrange("(c p) t -> p c t", p=128), st[:])
            return
        # ---------------- Phase E: MLP token-parallel ----------------
        zpool = ctx.enter_context(tc.tile_pool(name="zpool", bufs=1))
        z_sb = zpool.tile([128, 32, TCH], BF16, tag="z")
        with tc.tile_pool(name="w1pool", bufs=2) as w1pool, \
             tc.tile_pool(name="ewk", bufs=3) as ewk, \
             tc.tile_pool(name="epsum", bufs=4, space="PSUM") as epsum:
            for half in range(2):
                f1h = w1pool.tile([128, 8, 2048], BF16, tag="f1h",
                                  name=f"f1h{half}")
                nc.sync.dma_start(
                    f1h[:],
                    fc1t.ap().rearrange("(c p) m -> p c m", p=128)[
                        :, :, half * 2048:(half + 1) * 2048])
                for mp in range(8):
                    mcs = [16 * half + 2 * mp + i for i in range(2)]
                    pss = [epsum.tile([128, TCH], F32, tag="z1ps",
                                      name=f"z1_{mc}") for mc in mcs]
                    for f in range(8):
                        for i in range(2):
                            m = 2 * mp + i
                            nc.tensor.matmul(
                                pss[i][:], f1h[:, f, m * 128:(m + 1) * 128],
                                h_sb[:, f, :], start=(f == 0), stop=(f == 7))
                    for i, mc in enumerate(mcs):
                        sg = ewk.tile([128, TCH], F32, tag="sg", name=f"sg{mc}")
                        nc.scalar.activation(sg[:], pss[i][:], AF.Sigmoid,
                                             bias=fc1b_sb[:, mc:mc + 1])
                        nc.vector.scalar_tensor_tensor(
                            z_sb[:, mc, :], pss[i][:], fc1b_sb[:, mc:mc + 1],
                            sg[:], OP.add, OP.mult)

        fpool = ctx.enter_context(tc.tile_pool(name="fpool", bufs=1))
        mixed_sb = fpool.tile([128, 8, TCH], F32, tag="mixed")
        with tc.tile_pool(name="w2pool", bufs=2) as w2pool, \
             tc.tile_pool(name="mpsum", bufs=8, space="PSUM") as mpsum:
            mps = [mpsum.tile([128, TCH], F32, tag="mps", name=f"mps{oc}")
                   for oc in range(8)]
            for half in range(2):
                f2h = w2pool.tile([128, 16, H], BF16, tag="f2h",
                                  name=f"f2h{half}")
                nc.sync.dma_start(
                    f2h[:],
                    fc2t.ap()[half * 2048:(half + 1) * 2048, :].rearrange(
                        "(c p) o -> p c o", p=128))
                for kc in range(16):
                    for oc in range(8):
                        nc.tensor.matmul(
                            mps[oc][:], f2h[:, kc, oc * 128:(oc + 1) * 128],
                            z_sb[:, 16 * half + kc, :],
                            start=(half == 0 and kc == 0),
                            stop=(half == 1 and kc == 15))
            for oc in range(8):
                nc.scalar.activation(mixed_sb[:, oc, :], mps[oc][:], AF.Identity,
                                     bias=fc2b_sb[:, oc:oc + 1])

        if "F" not in phases:
            nc.sync.dma_start(out.ap().rearrange("(c p) t -> p c t", p=128), mixed_sb[:])
            return
        # ---------------- Phase F: RMSNorm -> upd -> shortcut ----------------
        with tc.tile_pool(name="fwk", bufs=1) as fwk, \
             tc.tile_pool(name="fpsum", bufs=4, space="PSUM") as fpsum, \
             tc.tile_pool(name="sqp", bufs=1, space="PSUM") as sqp:
            msq_sb = fwk.tile([128, 8, TCH], F32, tag="msq")
            nc.scalar.activation(msq_sb[:], mixed_sb[:], AF.Square)
            ssq = sqp.tile([1, TCH], F32, tag="ssq")
            for c in range(8):
                nc.tensor.matmul(ssq[:], ones1_sb[:], msq_sb[:, c, :],
                                 start=(c == 0), stop=(c == 7))
            srow = fwk.tile([1, TCH], F32, tag="srow")
            nc.scalar.activation(srow[:], ssq[:], AF.Sqrt,
                                 bias=eps_sb[:], scale=1.0 / H)
            rrow = fwk.tile([1, TCH], F32, tag="rrow")
            nc.vector.reciprocal(rrow[:], srow[:])
            rb = fwk.tile([128, TCH], F32, tag="rb")
            nc.gpsimd.partition_broadcast(rb[:], rrow[:])
            pp_sb = fwk.tile([128, 8, TCH], BF16, tag="pp")
            nc.vector.tensor_tensor(
                pp_sb[:], mixed_sb[:], rb[:, None, :].broadcast_to([128, 8, TCH]),
                OP.mult)
            updt_sb = fwk.tile([128, 8, H], BF16, tag="updt")
            nc.sync.dma_start(updt_sb[:], updt.ap().rearrange("(c p) m -> p c m", p=128))
            sct_sb = fwk.tile([128, 8, H], BF16, tag="sct")
            nc.sync.dma_start(sct_sb[:], sct.ap().rearrange("(c p) m -> p c m", p=128))
            s_sb = fwk.tile([128, 8, TCH], BF16, tag="s")
            for op_ in range(4):
                ocs = [2 * op_ + i for i in range(2)]
                pss = [fpsum.tile([128, TCH], F32, tag="fps", name=f"u{oc}")
                       for oc in ocs]
                for f in range(8):
                    for i, oc in enumerate(ocs):
                        nc.tensor.matmul(
                            pss[i][:], updt_sb[:, f, oc * 128:(oc + 1) * 128],
                            pp_sb[:, f, :], start=(f == 0), stop=(f == 7))
                for i, oc in enumerate(ocs):
                    nc.vector.scalar_tensor_tensor(
                        s_sb[:, oc, :], pss[i][:], updb_sb[:, oc:oc + 1],
                        mixed_sb[:, oc, :], OP.add, OP.add)
            out_sb = fwk.tile([128, 8, TCH], F32, tag="outsb")
            for op_ in range(4):
                ocs = [2 * op_ + i for i in range(2)]
                pss = [fpsum.tile([128, TCH], F32, tag="fps", name=f"sc{oc}")
                       for oc in ocs]
                for f in range(8):
                    for i, oc in enumerate(ocs):
                        nc.tensor.matmul(
                            pss[i][:], sct_sb[:, f, oc * 128:(oc + 1) * 128],
                            s_sb[:, f, :], start=(f == 0), stop=(f == 7))
                for i, oc in enumerate(ocs):
                    nc.vector.scalar_tensor_tensor(
                        out_sb[:, oc, :], pss[i][:], scb_sb[:, oc:oc + 1],
                        xc_sb[:, oc, :], OP.add, OP.add)
            nc.sync.dma_start(out.ap().rearrange("(c p) t -> p c t", p=128), out_sb[:])


# ---------------------------------------------------------------------------
# Host-side sharding / gather
# ---------------------------------------------------------------------------

def _eo_cols(w_qk_head):
    return np.concatenate([w_qk_head[0::2], w_qk_head[1::2]], axis=0)


def make_in_maps(x, qkv_w, out_w, fc1_w, fc1_b, fc2_w, fc2_b, norm_w,
                 upd_w, upd_b, sc_w, sc_b):
    x = np.asarray(x, np.float32)
    qkv_w = np.asarray(qkv_w, np.float32)
    out_w = np.asarray(out_w, np.float32)
    fc1_w = np.asarray(fc1_w, np.float32)
    fc2_w = np.asarray(fc2_w, np.float32)
    norm_w = np.asarray(norm_w, np.float32)
    upd_w = np.asarray(upd_w, np.float32)
    sc_w = np.asarray(sc_w, np.float32)
    qw = qkv_w[0:H].reshape(HEADS, HD, H)
    kw = qkv_w[H:2 * H].reshape(HEADS, HD, H)
    vw = qkv_w[2 * H:3 * H].reshape(HEADS, HD, H)

    d = np.arange(0, HD, 2, dtype=np.float32) / HD
    inv_freq = 1.0 / (ROPE_THETA ** d)
    tpos = np.arange(S, dtype=np.float32)
    freqs = tpos[None, :] * inv_freq[:, None]
    cosf = np.tile(np.cos(freqs), (4, 1)).astype(NP_BF16)
    sinf = np.tile(np.sin(freqs), (4, 1)).astype(NP_BF16)

    def bcol(v, ncol):
        return np.ascontiguousarray(
            np.asarray(v, np.float32).reshape(ncol, 128).T)

    def bf(a):
        return np.ascontiguousarray(np.asarray(a).astype(NP_BF16))

    updw_folded = upd_w * norm_w[None, :]

    shared = {
        "fc1t": bf(fc1_w.T),
        "fc1b": bcol(np.asarray(fc1_b, np.float32), 32),
        "fc2t": bf(fc2_w.T),
        "fc2b": bcol(fc2_b, 8),
        "updt": bf(updw_folded.T),
        "updb": bcol(upd_b, 8),
        "sct": bf(sc_w.T),
        "scb": bcol(sc_b, 8),
        "cosf": cosf,
        "sinf": sinf,
    }

    in_maps = []
    for c in range(NCORES):
        g, r = c // TPW, c % TPW
        heads = [4 * r + i for i in range(HL)]
        cols = []
        for w, scale in ((qw, 0.125), (kw, 1.0)):
            for j in range(2):
                hA, hB = heads[2 * j], heads[2 * j + 1]
                blk = np.concatenate([_eo_cols(w[hA]), _eo_cols(w[hB])],
                                     axis=0) * scale
                cols.append(blk)
        qkt = np.concatenate(cols, axis=0).T
        vwt = np.concatenate([vw[h] for h in heads], axis=0).T
        in_maps.append(dict(shared,
            xt=bf(x[g].T),
            xc=np.ascontiguousarray(x[g][TCH * r:TCH * (r + 1), :].T),
            qkt=bf(qkt),
            vwt=bf(vwt),
            owt=bf(out_w[:, 256 * r:256 * (r + 1)].T),
        ))
    return in_maps


_inmap_cache = {}


def _cached_in_maps(inputs):
    key = tuple(id(v) for _, v in sorted(inputs.items()))
    hit = _inmap_cache.get(key)
    if hit is not None:
        return hit[0]
    in_maps = make_in_maps(**inputs)
    # keep the input arrays alive so ids stay valid
    _inmap_cache.clear()
    _inmap_cache[key] = (in_maps, list(inputs.values()))
    return in_maps


def run(inputs, trace=False, reps=1, **kw):
    nc = build_program(reps)
    in_maps = _cached_in_maps(inputs)
    res = run_bass_kernel_spmd(nc, in_maps, list(range(NCORES)), trace=trace, **kw)
    outs = np.empty((B, S, H), np.float32)
    for c in range(NCORES):
        g, r = c // TPW, c % TPW
        outs[g, TCH * r:TCH * (r + 1), :] = res.results[c]["out"].T
    return outs, res


def kernel(**inputs):
    outs, _ = run(inputs)
    return outs
